# revision 8
# baseline (speedup 1.0000x reference)
"""Self-contained Trainium2 Bass kernel for the FCOS detection head problem.

Sharding: 8 cores = 2 images x 4 H-quarters (every level split the same way).
Each core runs an identical SPMD program: 4-layer cls tower (fp32) + score/ctr
head, 4-layer box tower (fp32r) + pred head, over halo-padded slabs of all 5
FPN levels. Host does the final (cheap) top-k/NMS selection tail.
"""
import numpy as np
from contextlib import ExitStack

import concourse.bass as bass
import concourse.bacc as bacc
import concourse.tile as tile
import concourse.mybir as mybir
from concourse.bass_utils import run_bass_kernel_spmd

F32 = mybir.dt.float32
F32R = mybir.dt.float32r

# ----- problem geometry (hardcoded; must match the grader's reference)
NUM_CLASSES = 80
STRIDES = (8, 16, 32, 64, 128)
LVL_HW = ((128, 168), (64, 84), (32, 42), (16, 21), (8, 11))
IMG_H, IMG_W = 1024, 1344
SCORE_T = 0.05
TOPK = 1000
NMS_T = 0.6
MAX_DET = 100
SCALE_CLAMP = float(np.log(1000.0 / 16.0))

NQ = 4                                    # quarters per image
HQ = [h // NQ for h, w in LVL_HW]          # valid rows per quarter: 32,16,8,4,2
RQ = [h + 10 for h in HQ]                  # slab rows (5 halo each side)
WP = [w + 2 for h, w in LVL_HW]            # padded width
MARG = [w + 2 for w in WP]                 # per-level margin elems
SLAB = [RQ[i] * WP[i] for i in range(5)]   # slab elems per level
SLABTOT = sum(SLAB)                        # 10646
# buffer offsets (margins around each level's slab)
BUF_OFF = []
_o = 0
for i in range(5):
    _o += MARG[i]
    BUF_OFF.append(_o)
    _o += SLAB[i] + MARG[i]
BUFTOT = _o

# head chunk layout: chunks of 128 over each level's valid span [5*W', 5*W'+h*W')
HEAD_CHUNKS = []  # (level, slab_start)
for l in range(5):
    s0 = 5 * WP[l]
    span = HQ[l] * WP[l]
    n = -(-span // 128)
    for k in range(n):
        st = s0 + k * 128
        if st + 128 > s0 + span:
            st = max(0, min(s0 + span - 128, SLAB[l] - 128))
        HEAD_CHUNKS.append((l, st))
NCHUNK = len(HEAD_CHUNKS)                 # 59
# pred head span tiles (N<=512 within each level's span)
PRED_TILES = []  # (level, slab_start, n)
SPAN_OFF = []
_so = 0
for l in range(5):
    SPAN_OFF.append(_so)
    span = HQ[l] * WP[l]
    s0 = 5 * WP[l]
    p = 0
    while p < span:
        n = min(512, span - p)
        PRED_TILES.append((l, s0 + p, n))
        p += n
    _so += span
SPANTOT = _so                             # 7286

EM_OFF = []
_e = 0
for _l in range(5):
    EM_OFF.append(_e)
    _e += 10 * WP[_l]
EMTOT = _e

_CACHE = {}
PROFILE = False


def _build_program():
    nc = bacc.Bacc("TRN2", target_bir_lowering=False, debug=False, num_devices=8)

    def din(name, shape, dt=F32):
        return nc.dram_tensor(name, list(shape), dt, kind="ExternalInput").ap()

    def dout(name, shape, dt=F32):
        return nc.dram_tensor(name, list(shape), dt, kind="ExternalOutput").ap()

    xin = din("xin", [2, 128, SLABTOT])
    xin_r = din("xin_r", [2, 128, SLABTOT], F32R)
    wcls = din("wcls", [4, 128, 36 * 128])
    wbox = din("wbox", [4, 128, 36 * 128])
    whead = din("whead", [128, 18 * 81])
    wpred = din("wpred", [128, 18 * 4])
    bcls = din("bcls", [128, 8])
    bbox = din("bbox", [128, 8])
    bhead = din("bhead", [128, 81])
    emask = din("emask", [128, EMTOT])
    emask_r = din("emask_r", [128, EMTOT], F32R)      # per level: 10 rows (5 top + 5 bottom)

    cls_out = dout("cls_out", [128, NCHUNK * 81])
    deltas_out = dout("deltas_out", [4, SPANTOT])

    TAPS = [(dy, dx) for dy in (-1, 0, 1) for dx in (-1, 0, 1)]
    MAXBUF = max(SLAB[l] + 2 * MARG[l] for l in range(5))

    with tile.TileContext(nc) as tc, ExitStack() as ctx:
        big = ctx.enter_context(tc.tile_pool(name="big", bufs=1))
        wpool = ctx.enter_context(tc.tile_pool(name="wpool", bufs=2))
        cpool = ctx.enter_context(tc.tile_pool(name="cpool", bufs=1))
        spool = ctx.enter_context(tc.tile_pool(name="spool", bufs=3))
        psum = ctx.enter_context(tc.tile_pool(name="psum", bufs=4, space="PSUM"))
        hpsum = ctx.enter_context(tc.tile_pool(name="hpsum", bufs=2, space="PSUM"))

        em = cpool.tile([128, EMTOT], F32)
        nc.sync.dma_start(em[:], emask)
        em_r = cpool.tile([128, EMTOT], F32R)
        nc.sync.dma_start(em_r[:], emask_r)
        bc = cpool.tile([128, 8], F32)
        nc.sync.dma_start(bc[:], bcls)
        bb = cpool.tile([128, 8], F32)
        nc.sync.dma_start(bb[:], bbox)
        bh = cpool.tile([128, 81], F32)
        nc.sync.dma_start(bh[:], bhead)
        wh = cpool.tile([128, 18 * 81], F32)
        nc.sync.dma_start(wh[:], whead)
        wpr = cpool.tile([128, 18 * 4], F32)
        nc.sync.dma_start(wpr[:], wpred)

        # chunk index within level for cls_out ordering
        lvl_chunk0 = {}
        _k = 0
        for l in range(5):
            lvl_chunk0[l] = _k
            _k += sum(1 for (ll, _s) in HEAD_CHUNKS if ll == l)

        for tower in range(2):          # 0 = cls (f32), 1 = box (f32r)
            is_box = (tower == 1)
            r_mode = False  # run both towers in fp32 (fp32r hits codegen limits)
            dt_t = F32R if r_mode else F32
            wdram = wbox if is_box else wcls
            bias_sb = bb if is_box else bc
            xin_src = xin_r if r_mode else xin
            em_t = em_r if r_mode else em
            for l in range(5):
                base = MARG[l]
                wl = WP[l]
                R = RQ[l]
                h = HQ[l]
                nbuf = SLAB[l] + 2 * MARG[l]
                T = [[big.tile([128, MAXBUF], dt_t, tag=f"t{s}c{c}", name=f"T{s}c{c}_{tower}_{l}")
                      for c in range(2)] for s in range(2)]
                off_xin = sum(SLAB[:l])
                for c in range(2):
                    nc.vector.memset(T[0][c][:, 0:nbuf], 0.0)
                    nc.vector.memset(T[1][c][:, 0:nbuf], 0.0)
                    nc.sync.dma_start(T[0][c][:, base:base + SLAB[l]],
                                      xin_src[c, :, off_xin:off_xin + SLAB[l]])
                for layer in range(4):
                    src = T[layer % 2]
                    dst = T[(layer + 1) % 2]
                    w_sb = wpool.tile([128, 36 * 128], dt_t, tag="w")
                    nc.sync.dma_start(w_sb[:], wdram[layer])
                    pos = 0
                    while pos < SLAB[l]:
                        n = min(512, SLAB[l] - pos)
                        for oc in range(2):
                            ps = psum.tile([128, 512], F32, tag="ps")
                            mi = 0
                            for ti, (dy, dx) in enumerate(TAPS):
                                sh = dy * wl + dx
                                for ic in range(2):
                                    widx = (ti * 2 + ic) * 2 + oc
                                    lhsT = w_sb[:, widx * 128:(widx + 1) * 128]
                                    rhs = src[ic][:, base + pos + sh:base + pos + sh + n]
                                    nc.tensor.matmul(ps[:, 0:n], lhsT, rhs,
                                                     start=(mi == 0), stop=(mi == 35))
                                    mi += 1
                            nc.scalar.activation(dst[oc][:, base + pos:base + pos + n],
                                                 ps[:, 0:n],
                                                 mybir.ActivationFunctionType.Relu,
                                                 bias=bias_sb[:, layer * 2 + oc:layer * 2 + oc + 1])
                        pos += n
                    emo = EM_OFF[l]
                    for c in range(2):
                        # zero pad columns (col 0 and col Wl+1 of every slab row)
                        slabview = dst[c][:, base:base + SLAB[l]].rearrange("p (r w) -> p r w", w=wl)
                        nc.vector.memset(slabview[:, :, 0:1], 0.0)
                        nc.vector.memset(slabview[:, :, LVL_HW[l][1] + 1:LVL_HW[l][1] + 2], 0.0)
                        # edge-row masks (top 5 rows, bottom 5 rows)
                        nc.vector.tensor_mul(dst[c][:, base:base + 5 * wl],
                                             dst[c][:, base:base + 5 * wl],
                                             em_t[:, emo:emo + 5 * wl])
                        nc.vector.tensor_mul(dst[c][:, base + (R - 5) * wl:base + R * wl],
                                             dst[c][:, base + (R - 5) * wl:base + R * wl],
                                             em_t[:, emo + 5 * wl:emo + 10 * wl])
                x4 = T[0]
                if not is_box:
                    # score + ctr head for this level
                    k0 = lvl_chunk0[l]
                    for kk, (ll, st) in enumerate(HEAD_CHUNKS):
                        if ll != l:
                            continue
                        ps = hpsum.tile([128, 81], F32, tag="hps")
                        mi = 0
                        for ti, (dy, dx) in enumerate(TAPS):
                            sh = dy * wl + dx
                            for ic in range(2):
                                widx = ti * 2 + ic
                                nc.tensor.matmul(ps[:], x4[ic][:, base + st + sh:base + st + sh + 128],
                                                 wh[:, widx * 81:(widx + 1) * 81],
                                                 start=(mi == 0), stop=(mi == 17))
                                mi += 1
                        stg = spool.tile([128, 81], F32, tag="stg")
                        nc.vector.tensor_add(stg[:], ps[:], bh[:])
                        nc.sync.dma_start(cls_out[:, kk * 81:(kk + 1) * 81], stg[:])
                else:
                    # pred head for this level
                    for (ll, st, n) in PRED_TILES:
                        if ll != l:
                            continue
                        sp = SPAN_OFF[l] + (st - 5 * wl)
                        ps = hpsum.tile([4, 512], F32, tag="pps")
                        mi = 0
                        for ti, (dy, dx) in enumerate(TAPS):
                            sh = dy * wl + dx
                            for ic in range(2):
                                widx = ti * 2 + ic
                                nc.tensor.matmul(ps[0:4, 0:n],
                                                 wpr[:, widx * 4:(widx + 1) * 4],
                                                 x4[ic][:, base + st + sh:base + st + sh + n],
                                                 start=(mi == 0), stop=(mi == 17))
                                mi += 1
                        stg = spool.tile([4, 512], F32, tag="pstg")
                        nc.vector.tensor_copy(stg[0:4, 0:n], ps[0:4, 0:n])
                        nc.sync.dma_start(deltas_out[:, sp:sp + n], stg[0:4, 0:n])

    nc.compile()
    return nc


# ---------------- host side ----------------

def _prep_inputs(inputs):
    """Build per-core in_maps."""
    feats = [inputs[f"feat{i}"] for i in range(5)]
    cls_w, cls_b = inputs["cls_w"], inputs["cls_b"]
    box_w, box_b = inputs["box_w"], inputs["box_b"]
    score_w, score_b = inputs["score_w"], inputs["score_b"]
    pred_w, pred_b = inputs["pred_w"], inputs["pred_b"]
    ctr_w, ctr_b = inputs["ctr_w"], inputs["ctr_b"]

    def pack_tower(w):  # w: (4,256,256,3,3) -> [4,128,36*128] lhsT=(ic,oc) per tap
        out = np.zeros((4, 128, 36 * 128), np.float32)
        for layer in range(4):
            for ti in range(9):
                dy, dx = ti // 3, ti % 3
                for ic in range(2):
                    for oc in range(2):
                        widx = (ti * 2 + ic) * 2 + oc
                        blk = w[layer, oc * 128:(oc + 1) * 128, ic * 128:(ic + 1) * 128, dy, dx]
                        out[layer, :, widx * 128:(widx + 1) * 128] = blk.T
        return out

    wcls = pack_tower(cls_w)
    wbox = pack_tower(box_w)

    whead = np.zeros((128, 18 * 81), np.float32)
    hw_all = np.concatenate([score_w, ctr_w], axis=0)  # (81,256,3,3)
    for ti in range(9):
        dy, dx = ti // 3, ti % 3
        for ic in range(2):
            widx = ti * 2 + ic
            whead[:, widx * 81:(widx + 1) * 81] = hw_all[:, ic * 128:(ic + 1) * 128, dy, dx].T
    wpred = np.zeros((128, 18 * 4), np.float32)
    for ti in range(9):
        dy, dx = ti // 3, ti % 3
        for ic in range(2):
            widx = ti * 2 + ic
            wpred[:, widx * 4:(widx + 1) * 4] = pred_w[:, ic * 128:(ic + 1) * 128, dy, dx].T

    bclsa = np.zeros((128, 8), np.float32)
    bboxa = np.zeros((128, 8), np.float32)
    for layer in range(4):
        for oc in range(2):
            bclsa[:, layer * 2 + oc] = cls_b[layer, oc * 128:(oc + 1) * 128]
            bboxa[:, layer * 2 + oc] = box_b[layer, oc * 128:(oc + 1) * 128]
    bheada = np.tile(np.concatenate([score_b, ctr_b])[None, :], (128, 1)).astype(np.float32)

    in_maps = []
    for core in range(8):
        img, q = core // 4, core % 4
        xin = np.zeros((2, 128, SLABTOT), np.float32)
        emask = np.zeros((128, EMTOT), np.float32)
        for l in range(5):
            Hl, Wl = LVL_HW[l]
            h, R, wl = HQ[l], RQ[l], WP[l]
            r0 = q * h - 5
            slab = np.zeros((256, R, wl), np.float32)
            lo, hi = max(r0, 0), min(r0 + R, Hl)
            slab[:, lo - r0:hi - r0, 1:Wl + 1] = feats[l][img][:, lo:hi, :]
            off = sum(SLAB[:l])
            xin[0, :, off:off + SLAB[l]] = slab[:128].reshape(128, -1)
            xin[1, :, off:off + SLAB[l]] = slab[128:].reshape(128, -1)
            m = np.ones((10, wl), np.float32)
            m[:, 0] = 0.0
            m[:, Wl + 1:] = 0.0
            if q == 0:
                m[0:5, :] = 0.0
            if q == NQ - 1:
                m[5:, :] = 0.0
            emask[:, EM_OFF[l]:EM_OFF[l] + 10 * wl] = m.reshape(-1)[None, :]
        in_maps.append(dict(xin=xin, xin_r=xin, wcls=wcls, wbox=wbox, whead=whead,
                            wpred=wpred, bcls=bclsa, bbox=bboxa, bhead=bheada,
                            emask=emask, emask_r=emask))
    return in_maps


def _assemble(results):
    """Reconstruct per-image per-level logits/ctr/deltas at valid locations."""
    logits = [np.zeros((2, LVL_HW[l][0] * LVL_HW[l][1], NUM_CLASSES), np.float32) for l in range(5)]
    ctr = [np.zeros((2, LVL_HW[l][0] * LVL_HW[l][1], 1), np.float32) for l in range(5)]
    deltas = [np.zeros((2, LVL_HW[l][0] * LVL_HW[l][1], 4), np.float32) for l in range(5)]
    for core in range(8):
        img, q = core // 4, core % 4
        co = results[core]["cls_out"]          # [128, NCHUNK*81]
        dl = results[core]["deltas_out"]       # [4, SPANTOT]
        for k, (l, st) in enumerate(HEAD_CHUNKS):
            Hl, Wl = LVL_HW[l]
            h, wl = HQ[l], WP[l]
            pos = st + np.arange(128)
            row = pos // wl
            col = pos % wl
            valid = (row >= 5) & (row < 5 + h) & (col >= 1) & (col <= Wl)
            gr = q * h + row[valid] - 5
            gc = col[valid] - 1
            loc = gr * Wl + gc
            blk = co[:, k * 81:(k + 1) * 81]
            logits[l][img, loc, :] = blk[valid, :80]
            ctr[l][img, loc, 0] = blk[valid, 80]
        for l in range(5):
            Hl, Wl = LVL_HW[l]
            h, wl = HQ[l], WP[l]
            span = dl[:, SPAN_OFF[l]:SPAN_OFF[l] + h * wl].reshape(4, h, wl)
            gr = q * h + np.arange(h)
            deltas[l][img, gr[:, None] * Wl + np.arange(Wl)[None, :], :] = \
                span[:, :, 1:Wl + 1].transpose(1, 2, 0)
    return logits, ctr, deltas


def _anchors_lvl(h, w, stride):
    ys = (np.arange(h, dtype=np.float32) + 0.5) * stride
    xs = (np.arange(w, dtype=np.float32) + 0.5) * stride
    cy, cx = np.meshgrid(ys, xs, indexing="ij")
    cx, cy = cx.reshape(-1), cy.reshape(-1)
    half = np.float32(2.0 * stride)
    return np.stack([cx - half, cy - half, cx + half, cy + half], -1).astype(np.float32)


def _decode32(anc, d):
    wa = anc[:, 2] - anc[:, 0]
    ha = anc[:, 3] - anc[:, 1]
    cxa = anc[:, 0] + np.float32(0.5) * wa
    cya = anc[:, 1] + np.float32(0.5) * ha
    dx, dy = d[:, 0], d[:, 1]
    dw = np.minimum(d[:, 2], np.float32(SCALE_CLAMP))
    dh = np.minimum(d[:, 3], np.float32(SCALE_CLAMP))
    cx = dx * wa + cxa
    cy = dy * ha + cya
    w = np.exp(dw) * wa
    h = np.exp(dh) * ha
    x1 = np.clip(cx - np.float32(0.5) * w, 0, IMG_W).astype(np.float32)
    y1 = np.clip(cy - np.float32(0.5) * h, 0, IMG_H).astype(np.float32)
    x2 = np.clip(cx + np.float32(0.5) * w, 0, IMG_W).astype(np.float32)
    y2 = np.clip(cy + np.float32(0.5) * h, 0, IMG_H).astype(np.float32)
    return np.stack([x1, y1, x2, y2], -1)


def _select_nms_image(per_level):
    all_b, all_s, all_c = [], [], []
    for lvl, (sc, dd) in enumerate(per_level):
        flat = sc.reshape(-1)
        k = min(TOPK, flat.shape[0])
        topi = np.argsort(-flat, kind="stable")[:k]
        topv = flat[topi]
        topv = np.where(topv > np.float32(SCORE_T), topv, np.float32(0.0))
        aidx = topi // NUM_CLASSES
        cidx = (topi % NUM_CLASSES).astype(np.int32)
        anc = _anchors_lvl(*LVL_HW[lvl], STRIDES[lvl])[aidx]
        all_b.append(_decode32(anc, dd[aidx]))
        all_s.append(topv)
        all_c.append(cidx)
    boxes = np.concatenate(all_b, 0)
    scores = np.concatenate(all_s, 0)
    classes = np.concatenate(all_c, 0)
    M = boxes.shape[0]
    b = boxes + (classes.astype(np.float32) * np.float32(2000.0))[:, None]
    order = np.argsort(-scores, kind="stable")
    b = b[order]
    s = scores[order]
    areas = (b[:, 2] - b[:, 0]) * (b[:, 3] - b[:, 1])
    keep = np.ones(M, bool)
    idx = np.arange(M)
    for i in range(M):
        if not (keep[i] and s[i] > 0):
            continue
        xx1 = np.maximum(b[i, 0], b[:, 0])
        yy1 = np.maximum(b[i, 1], b[:, 1])
        xx2 = np.minimum(b[i, 2], b[:, 2])
        yy2 = np.minimum(b[i, 3], b[:, 3])
        inter = np.clip(xx2 - xx1, 0, None) * np.clip(yy2 - yy1, 0, None)
        iou = inter / (areas[i] + areas - inter + np.float32(1e-9))
        keep &= ~((iou > NMS_T) & (idx > i))
        keep[i] = True
    kept = np.where(keep, s, np.float32(0.0))
    topi = np.argsort(-kept, kind="stable")[:MAX_DET]
    topv = kept[topi].astype(np.float32)
    sel = order[topi]
    return boxes[sel], topv, classes[sel]


def kernel(**inputs):
    if "nc" not in _CACHE:
        _CACHE["nc"] = _build_program()
    nc = _CACHE["nc"]
    in_maps = _prep_inputs(inputs)
    res = run_bass_kernel_spmd(nc, in_maps, list(range(8)), trace=PROFILE)
    _CACHE["last_results"] = res
    logits, ctr, deltas = _assemble(res.results)
    out_b, out_s, out_c = [], [], []
    for img in range(2):
        per_level = []
        for l in range(5):
            sig_l = 1.0 / (1.0 + np.exp(-logits[l][img].astype(np.float32)))
            sig_c = 1.0 / (1.0 + np.exp(-ctr[l][img].astype(np.float32)))
            sc = np.sqrt(sig_l * sig_c).astype(np.float32)
            per_level.append((sc, deltas[l][img]))
        b, s, c = _select_nms_image(per_level)
        out_b.append(b)
        out_s.append(s)
        out_c.append(c)
    return (np.stack(out_b).astype(np.float32), np.stack(out_s).astype(np.float32),
            np.stack(out_c).astype(np.int32))
